# revision 64
# baseline (speedup 1.0000x reference)
"""Bass/Tile TRN2 kernel for EnhancedIPA3 — collective-free redesign.

8 cores = batch(2) x query-block(4).  Each core redundantly computes the
K/V-side features for ALL 1024 keys of its batch (projections + rigid
frame transforms), then runs attention for its own 256 queries only.  No
inter-core communication: the collective-bootstrap barrier and the two
serialized AllGathers of the previous design are gone, and the cores are
fully independent, so cross-core launch skew no longer costs anything.

Key rows are rotated per core so the core's own query rows are always
tiles 0..1 (softmax over keys is permutation invariant) — one SPMD
program serves all 8 cores.

Self-contained: hardcodes all shapes; only depends on numpy + concourse.
"""

import numpy as np
from contextlib import ExitStack

import concourse.bass as bass
import concourse.bacc as bacc
import concourse.mybir as mybir
import concourse.tile as tile
from concourse.bass_utils import run_bass_kernel_spmd
from concourse.masks import make_identity

F32 = mybir.dt.float32
F32R = mybir.dt.float32r
BF16 = mybir.dt.bfloat16
AF = mybir.ActivationFunctionType
OP = mybir.AluOpType
AX = mybir.AxisListType

B, N, CS, H, C, P, V = 2, 1024, 384, 12, 16, 4, 8
EPS = 1e-8
NB = 256               # query rows per core (2 tiles)
NKT = 8                # key tiles of 128
# wall column map
K_OFF, V_OFF, PTS_OFF, Q_OFF, G_OFF, QPTS_OFF = 0, 192, 384, 1248, 1440, 1488
WALL_COLS = 1776
NPK = 12               # kv points per head (0:4 k_pts, 4:12 v_pts)
FEAT = 64              # per-head feature stride in kf/qf
FS = 42                # live features per head
OCH = 68               # va per-head stride: v16 | pts48 | ones | pad3
FOUT = H * (C + 7 * V)  # 864
KCH = 7                # output-proj contraction chunks


def _host_prep(inputs):
    wq = np.asarray(inputs["wq"], np.float32)
    bq = np.asarray(inputs["bq"], np.float32)
    wkv = np.asarray(inputs["wkv"], np.float32)
    bkv = np.asarray(inputs["bkv"], np.float32)
    wqp = np.asarray(inputs["wqp"], np.float32)
    bqp = np.asarray(inputs["bqp"], np.float32)
    wkvp = np.asarray(inputs["wkvp"], np.float32)
    bkvp = np.asarray(inputs["bkvp"], np.float32)
    wg = np.asarray(inputs["wg"], np.float32)
    bg = np.asarray(inputs["bg"], np.float32)
    gw = np.asarray(inputs["geom_weight"], np.float32)
    hw = np.asarray(inputs["head_weights"], np.float32)
    sh = 1.0 / (1.0 + np.exp(-hw))
    gw0, gw1 = float(gw[0]), float(gw[1])

    wall = np.zeros((CS + 2, WALL_COLS), np.float32)
    wall[:CS, K_OFF:K_OFF + 192] = wkv[:, :192]
    wall[CS, K_OFF:K_OFF + 192] = bkv[:192]
    wall[:CS, V_OFF:V_OFF + 192] = wkv[:, 192:]
    wall[CS, V_OFF:V_OFF + 192] = bkv[192:]
    # kv pts pair-planar: block 2j+t (j = input col 0..2, t: 0=coord 1=dir)
    # dst col = PTS_OFF + block*144 + h*12 + p  <-  src h*72 + p*6 + cc
    cc, h, p = np.meshgrid(np.arange(6), np.arange(H), np.arange(12),
                           indexing="ij")
    blk = np.where(cc < 3, 2 * cc, 2 * (cc - 3) + 1)
    dst = (PTS_OFF + blk * 144 + h * 12 + p).ravel()
    src = (h * 72 + p * 6 + cc).ravel()
    wall[:CS, dst] = wkvp[:, src]
    wall[CS, dst] = bkvp[src]
    # q scaled by sh/sqrt(C)
    qs = np.repeat(sh / np.sqrt(C), 16)
    wall[:CS, Q_OFF:Q_OFF + 192] = wq * qs[None, :]
    wall[CS, Q_OFF:Q_OFF + 192] = bq * qs
    wall[:CS, G_OFF:G_OFF + 48] = wg
    wall[CS, G_OFF:G_OFF + 48] = bg
    # q pts pair-planar: dst col = QPTS_OFF + blk*48 + h*4 + p
    cc, h, p = np.meshgrid(np.arange(6), np.arange(H), np.arange(P),
                           indexing="ij")
    blk = np.where(cc < 3, 2 * cc, 2 * (cc - 3) + 1)
    dst = (QPTS_OFF + blk * 48 + h * 4 + p).ravel()
    src = (h * 24 + p * 6 + cc).ravel()
    wall[:CS, dst] = wqp[:, src]
    wall[CS, dst] = bqp[src]
    has_bias = bool(np.any(wall[CS] != 0.0))
    wall[CS + 1] = wall[CS] * 0.5
    wall[CS] = wall[CS + 1]

    bout_half = np.asarray(inputs["bout"], np.float32)[None, :] * 0.5
    wout_b = np.concatenate(
        [np.asarray(inputs["wout"], np.float32), bout_half, bout_half], axis=0)

    # on-chip constants (broadcast to 128 partitions by a rank-1 matmul)
    qconst = np.zeros((1, 144), np.float32)
    SC = gw0 * 0.5 * sh            # coord feature scale (with gate)
    DC = gw1 * sh                  # dir feature scale (with gate)
    qconst[0, 0:48] = np.repeat(SC, P)
    qconst[0, 48:96] = np.repeat(DC, P)
    qconst[0, 96:108] = sh * gw0 / P                      # qf[40]
    c2 = np.where(np.abs(gw0 * sh) > 1e-12, -1.0 / (gw0 * sh + 1e-30), 0.0)
    qconst[0, 108:120] = c2                               # q2 coefficient
    qconst[0, 120:132] = -sh * gw1                        # curvature coeff

    rot9 = np.asarray(inputs["rot"], np.float32).reshape(B, N, 9)
    trans = np.asarray(inputs["trans"], np.float32)
    rt_all = np.ascontiguousarray(np.concatenate([rot9, trans], axis=2))
    s = np.asarray(inputs["s"], np.float32)
    return s, rt_all, wall, wout_b, qconst, has_bias


_PROGRAM_CACHE = {}


def _build_program(has_bias):
    key = (bool(has_bias),)
    if key in _PROGRAM_CACHE:
        return _PROGRAM_CACHE[key]
    nc = bacc.Bacc("TRN2", target_bir_lowering=False, debug=False,
                   num_devices=8)
    s_all = nc.dram_tensor("s_all", [N, CS], F32, kind="ExternalInput")
    rt_d = nc.dram_tensor("rt_all", [N, 12], F32, kind="ExternalInput")
    wall_d = nc.dram_tensor("wall", [CS + 2, WALL_COLS], F32,
                            kind="ExternalInput")
    wout_d = nc.dram_tensor("wout_b", [FOUT + 2, CS], F32,
                            kind="ExternalInput")
    qconst_d = nc.dram_tensor("qconst", [1, 144], F32, kind="ExternalInput")
    out_loc = nc.dram_tensor("out_loc", [NB, CS], F32, kind="ExternalOutput")

    with tile.TileContext(nc) as tc:
        with ExitStack() as ctx:
            _emit(ctx, tc, nc, s_all, rt_d, wall_d, wout_d,
                  qconst_d, out_loc, has_bias)
    nc.compile()
    _PROGRAM_CACHE[key] = nc
    return nc


def _emit(ctx, tc, nc, s_all, rt_d, wall_d, wout_d, qconst_d,
          out_loc, has_bias):
    PS = bass.MemorySpace.PSUM

    const = ctx.enter_context(tc.tile_pool(name="const", bufs=1))
    work = ctx.enter_context(tc.tile_pool(name="work", bufs=1))
    tmp = ctx.enter_context(tc.tile_pool(name="tmp", bufs=2))
    pre_ctx = ExitStack()
    pA = pre_ctx.enter_context(tc.tile_pool(name="pA", bufs=1))
    kio = pre_ctx.enter_context(tc.tile_pool(name="kio", bufs=2))
    ppsum = pre_ctx.enter_context(tc.tile_pool(name="ppsum", bufs=1, space=PS))
    tpsum = pre_ctx.enter_context(tc.tile_pool(name="tpsum", bufs=1, space=PS))

    # ---- constants -------------------------------------------------------
    ident = const.tile([128, 128], F32)
    make_identity(nc, ident[:])
    ident_r = const.tile([OCH, OCH], F32R)
    nc.gpsimd.tensor_copy(ident_r[:], ident[0:OCH, 0:OCH])
    ones2_f32 = const.tile([2, NB], F32)
    nc.gpsimd.memset(ones2_f32[:], 1.0)
    ones_r = const.tile([2, 128], F32R)
    nc.gpsimd.tensor_copy(ones_r[:], ones2_f32[:, 0:128])

    # ---- DMAs (s first: the transposes+projections are the critical path;
    # issue on multiple engine queues to parallelize descriptor setup)
    s_sb = []
    for kt in range(NKT):
        t = pA.tile([128, CS], F32, name=f"s{kt}")
        nc.sync.dma_start(t[:], s_all[kt * 128:(kt + 1) * 128, :])
        s_sb.append(t)
    # all rot+trans rows in one DMA: rt_sb[:, kt*12+c] = rt_all[kt*128+p, c]
    rt_sb = const.tile([128, 96], F32, name="rt_sb")
    nc.gpsimd.dma_start(rt_sb[:],
                        rt_d[:, :].rearrange("(t p) c -> p t c", p=128))
    rtb_sb = const.tile([128, 96], BF16, name="rtb_sb")
    nc.gpsimd.tensor_copy(rtb_sb[:], rt_sb[:])

    def Rc(kt, j, b=False):
        t = rtb_sb if b else rt_sb
        return t[:, kt * 12 + j:kt * 12 + j + 1]

    def Tc(kt, j, b=False):
        t = rtb_sb if b else rt_sb
        return t[:, kt * 12 + 9 + j:kt * 12 + 9 + j + 1]

    wall_sb = []
    for kc in range(3):
        t = pA.tile([128, WALL_COLS], F32R, name=f"wall{kc}")
        nc.sync.dma_start(t[:], wall_d[kc * 128:(kc + 1) * 128, :].bitcast(F32R))
        wall_sb.append(t)
    wall_bias = pA.tile([2, WALL_COLS], F32R)
    if has_bias:
        nc.sync.dma_start(wall_bias[:], wall_d[CS:CS + 2, :].bitcast(F32R))

    qconst_sb = const.tile([1, 144], F32R)
    nc.gpsimd.dma_start(qconst_sb[:], qconst_d[:, :].bitcast(F32R))

    wout_sb = []
    for kc in range(KCH):
        r0 = kc * 128
        r1 = min(FOUT + 2, r0 + 128)
        t = const.tile([r1 - r0, CS], F32R, name=f"wout{kc}")
        wout_sb.append(t)

    def emit_wout_dmas():
        for kc in range(KCH):
            r0 = kc * 128
            r1 = min(FOUT + 2, r0 + 128)
            nc.sync.dma_start(wout_sb[kc][:], wout_d[r0:r1, :].bitcast(F32R))

    # ---- sT (transpose all of s) ----------------------------------------
    sT = pA.tile([128, 3 * N], F32R, name="sT")   # [:, kc*1024 + key]
    sT3 = sT[:].rearrange("p (c k) -> p c k", k=N)
    for kt in range(NKT):
        tps = tpsum.tile([128, 384], F32, tag="tps")
        for kc in range(3):
            nc.tensor.transpose(tps[:, kc * 128:(kc + 1) * 128],
                                s_sb[kt][:, kc * 128:(kc + 1) * 128], ident[:])
        dst = sT3[:, :, kt * 128:(kt + 1) * 128]
        src = tps[:].rearrange("p (c k) -> p c k", k=128)
        if kt % 2:
            nc.scalar.copy(dst, src)
        else:
            nc.vector.tensor_copy(dst, src)

    # ---- broadcast qconst row to 128 partitions --------------------------
    tps = tpsum.tile([128, 384], F32, tag="tps")
    nc.tensor.matmul(tps[:, 0:144], ones_r[0:1, :], qconst_sb[:, :],
                     start=True, stop=True)
    qcst = const.tile([128, 144], F32)
    nc.vector.tensor_copy(qcst[:], tps[:, 0:144])
    # slices: SC48 0:48 | DC48 48:96 | A12 96:108 | c2 108:120 | c3 120:132

    # ---- K/V side: all 8 key tiles --------------------------------------
    kfT = work.tile([128, 6 * N], BF16, name="kfT")   # [:, t*1024 + key]
    kfT3 = kfT[:].rearrange("p (t k) -> p t k", k=N)
    vaG = [work.tile([128, H * OCH], BF16, name=f"vaG{kb}")
           for kb in range(NKT)]
    kds = [work.tile([128, 144], F32, name=f"kds{qt}") for qt in range(2)]

    GROUPS_K = [(0, 384), (384, 896), (896, 1248)]

    def proj_mm(ps, c0, c1, kt):
        pv = ps[:, 0:c1 - c0]
        for kc in range(3):
            last = (kc == 2) and not has_bias
            nc.tensor.matmul(pv, sT3[:, kc, kt * 128:(kt + 1) * 128],
                             wall_sb[kc][:, c0:c1], start=(kc == 0), stop=last)
        if has_bias:
            nc.tensor.matmul(pv, ones_r[:, :], wall_bias[:, c0:c1],
                             start=False, stop=True)

    def transform(pts, pco, kt, W):
        """Rigid transform, pair-planar [dc_j|dd_j] blocks of 2W.

        The coord and dir chains for output comp i share the same rotation
        column, so each chain step runs once on the fused [128, 2W] pair.
        """
        W2 = 2 * W
        for i in range(3):
            dco = pco[:, i * W2:(i + 1) * W2]
            nc.scalar.activation(dco, pts[:, 0:W2], AF.Copy,
                                 scale=Rc(kt, 3 * i))
            nc.vector.scalar_tensor_tensor(dco, pts[:, W2:2 * W2],
                                           Rc(kt, 3 * i + 1, True), dco,
                                           OP.mult, OP.add)
            nc.vector.scalar_tensor_tensor(dco, pts[:, 2 * W2:3 * W2],
                                           Rc(kt, 3 * i + 2, True), dco,
                                           OP.mult, OP.add)
            # + translation on the coord half only
            nc.scalar.activation(pco[:, i * W2:i * W2 + W],
                                 pco[:, i * W2:i * W2 + W], AF.Identity,
                                 bias=Tc(kt, i))

    def emit_ktile(kt):
        # projections: K+V | pts-a | pts-b
        ps_kv = ppsum.tile([128, 384], F32, tag="pg384", name="pskv")
        proj_mm(ps_kv, 0, 384, kt)
        ps_p1 = ppsum.tile([128, 512], F32, tag="pg512", name="psp1")
        proj_mm(ps_p1, 384, 896, kt)
        ps_p2 = ppsum.tile([128, 352], F32, tag="pg352", name="psp2")
        proj_mm(ps_p2, 896, 1248, kt)

        kf = kio.tile([128, H * FEAT], F32, tag="kf", name="kf", bufs=2)
        kfv = kf[:].rearrange("p (h f) -> p h f", f=FEAT)
        if kt < 2:
            # zero the pad cols 42:64 of this physical buffer once
            nc.gpsimd.memset(kfv[:, :, 42:64], 0.0)
        va = vaG[kt]
        vav = va[:].rearrange("p (h f) -> p h f", f=OCH)
        pts = kio.tile([128, 864], BF16, tag="pts", name="pts", bufs=2)

        # evacuations
        nc.scalar.copy(kfv[:, :, 0:16],
                       ps_kv[:, 0:192].rearrange("p (h c) -> p h c", c=16))
        nc.vector.tensor_copy(vav[:, :, 0:16],
                              ps_kv[:, 192:384].rearrange("p (h c) -> p h c", c=16))
        nc.scalar.activation(pts[:, 0:512], ps_p1[:], AF.Relu)
        nc.scalar.activation(pts[:, 512:864], ps_p2[:], AF.Relu)

        # rigid transform (planar, bf16)
        pco = kio.tile([128, 864], BF16, tag="pco", name="pco", bufs=2)
        transform(pts[:], pco[:], kt, 144)
        pco5 = pco[:].rearrange("p (j t h x) -> p j t h x", j=3, t=2, x=NPK)
        pco3 = pco[:].rearrange("p (c h x) -> p c h x", c=6, x=NPK)

        # kf coord/dir features ([cc*4+p] per head) + va pts (fused copies)
        nc.gpsimd.tensor_copy(
            kfv[:, :, 16:28].rearrange("p h (c x) -> p c h x", c=3),
            pco5[:, :, 0, :, 0:4])
        nc.gpsimd.tensor_copy(
            kfv[:, :, 28:40].rearrange("p h (c x) -> p c h x", c=3),
            pco5[:, :, 1, :, 0:4])
        nc.vector.tensor_copy(
            vav[:, :, 16:64].rearrange("p h (c x) -> p c h x", c=6),
            pco3[:, :, :, 4:12])
        nc.gpsimd.memset(vav[:, :, 64:65], 1.0)
        nc.gpsimd.memset(vav[:, :, 65:68], 0.0)

        # k2 (negated sum of squared coord features)
        sqs = tmp.tile([128, 144], F32, tag="sqs", name="sqs")
        nc.vector.tensor_tensor(
            sqs[:].rearrange("p (h x) -> p h x", x=12),
            kfv[:, :, 16:28], kfv[:, :, 16:28], OP.mult)
        nc.vector.tensor_reduce(
            kfv[:, :, 40], sqs[:].rearrange("p (h c x) -> p h c x", c=3, x=4),
            AX.XY, OP.add, negate=True)
        nc.gpsimd.memset(kfv[:, :, 41], 1.0)
        if kt < 2:
            nc.gpsimd.tensor_copy(kds[kt][:].rearrange("p (h x) -> p h x", x=12),
                                  kfv[:, :, 28:40])

        # transpose kf -> kfT (2 head-pairs per psum tile)
        for t0 in range(0, 6, 2):
            tps = tpsum.tile([128, 384], F32, tag="tps")
            nc.tensor.transpose(tps[:, 0:128],
                                kf[:, t0 * 128:(t0 + 1) * 128], ident[:])
            nc.tensor.transpose(tps[:, 128:256],
                                kf[:, (t0 + 1) * 128:(t0 + 2) * 128], ident[:])
            dst = kfT3[:, t0:t0 + 2, kt * 128:(kt + 1) * 128]
            src = tps[:, 0:256].rearrange("p (t k) -> p t k", k=128)
            if t0 == 2:
                nc.scalar.copy(dst, src)
            else:
                nc.vector.tensor_copy(dst, src)

    for _kt in range(2):
        emit_ktile(_kt)

    # ---- Q side (own rows = tiles 0..1) ---------------------------------
    qf_sb = [work.tile([128, H * FEAT], F32, name=f"qf{qt}") for qt in range(2)]
    for qt in range(2):
        qf = qf_sb[qt]
        qfv = qf[:].rearrange("p (h f) -> p h f", f=FEAT)
        ps_a = ppsum.tile([128, 384], F32, tag="pg384", name="psqa")
        proj_mm(ps_a, Q_OFF, Q_OFF + 384, qt)
        ps_b = ppsum.tile([128, 352], F32, tag="pg352", name="psqb")
        proj_mm(ps_b, Q_OFF + 384, WALL_COLS, qt)

        nc.scalar.copy(qfv[:, :, 0:16],
                       ps_a[:, 0:192].rearrange("p (h c) -> p h c", c=16))
        g_sb = tmp.tile([128, 48], F32, tag="gsb", name="gsb")
        nc.scalar.activation(g_sb[:], ps_a[:, 192:240], AF.Sigmoid)
        qpts = tmp.tile([128, 288], BF16, tag="qpts", name="qpts")
        nc.vector.tensor_scalar_max(qpts[:, 0:144], ps_a[:, 240:384], 0.0)
        nc.vector.tensor_scalar_max(qpts[:, 144:288], ps_b[:, 0:144], 0.0)

        qpco = tmp.tile([128, 288], F32, tag="qpco", name="qpco")
        transform(qpts[:], qpco[:], qt, 48)
        qpco5 = qpco[:].rearrange("p (j t h x) -> p j t h x", j=3, t=2, x=4)

        gc = tmp.tile([128, 48], F32, tag="gc", name="gc")
        gd = tmp.tile([128, 48], F32, tag="gd", name="gd")
        nc.vector.tensor_tensor(gc[:], g_sb[:], qcst[:, 0:48], OP.mult)
        nc.vector.tensor_tensor(gd[:], g_sb[:], qcst[:, 48:96], OP.mult)
        gc3 = gc[:].rearrange("p (h x) -> p h x", x=4)
        gd3 = gd[:].rearrange("p (h x) -> p h x", x=4)
        for cc in range(3):
            nc.vector.tensor_tensor(qfv[:, :, 16 + cc * 4:20 + cc * 4],
                                    qpco5[:, cc, 0], gc3, OP.mult)
            nc.gpsimd.tensor_tensor(qfv[:, :, 28 + cc * 4:32 + cc * 4],
                                    qpco5[:, cc, 1], gd3, OP.mult)
        nc.vector.tensor_copy(qfv[:, :, 40], qcst[:, 96:108])

        # q2 from coord features
        sqs = tmp.tile([128, 144], F32, tag="sqs", name="sqs")
        nc.vector.tensor_tensor(
            sqs[:].rearrange("p (h x) -> p h x", x=12),
            qfv[:, :, 16:28], qfv[:, :, 16:28], OP.mult)
        q2s = tmp.tile([128, 12], F32, tag="q2s", name="q2s")
        nc.vector.tensor_reduce(
            q2s[:], sqs[:].rearrange("p (h c x) -> p h c x", c=3, x=4),
            AX.XY, OP.add)

        # curvature from dir features vs raw kd features of same rows
        qdv = qfv[:, :, 28:40]
        kdv = kds[qt][:].rearrange("p (h x) -> p h x", x=12)
        crs = tmp.tile([128, 144], F32, tag="crs", name="crs")
        t1 = tmp.tile([128, 48], F32, tag="t1", name="t1")
        t2 = tmp.tile([128, 48], F32, tag="t2", name="t2")
        t13 = t1[:].rearrange("p (h x) -> p h x", x=4)
        t23 = t2[:].rearrange("p (h x) -> p h x", x=4)
        for c, (a, b2) in enumerate(((1, 2), (2, 0), (0, 1))):
            nc.vector.tensor_tensor(t13, qdv[:, :, a * 4:a * 4 + 4],
                                    kdv[:, :, b2 * 4:b2 * 4 + 4], OP.mult)
            nc.gpsimd.tensor_tensor(t23, qdv[:, :, b2 * 4:b2 * 4 + 4],
                                    kdv[:, :, a * 4:a * 4 + 4], OP.mult)
            nc.vector.tensor_tensor(crs[:, c * 48:(c + 1) * 48], t1[:], t2[:],
                                    OP.subtract)
        nc.vector.tensor_tensor(crs[:], crs[:], crs[:], OP.mult)
        csum = tmp.tile([128, 48], F32, tag="csum", name="csum")
        nc.vector.tensor_reduce(
            csum[:], crs[:].rearrange("p (c x) -> p x c", c=3), AX.X, OP.add)
        # |qfd|^2, |kd|^2 per (h,p)
        sqd = tmp.tile([128, 144], F32, tag="sqd", name="sqd")
        nq2 = tmp.tile([128, 48], F32, tag="nq2", name="nq2")
        nk2 = tmp.tile([128, 48], F32, tag="nk2", name="nk2")
        nc.gpsimd.tensor_tensor(sqd[:].rearrange("p (h x) -> p h x", x=12),
                                qdv, qdv, OP.mult)
        nc.vector.tensor_reduce(
            nq2[:].rearrange("p (h x) -> p h x", x=4),
            sqd[:].rearrange("p (h c x) -> p h x c", c=3, x=4), AX.X, OP.add)
        nc.gpsimd.tensor_tensor(sqd[:].rearrange("p (h x) -> p h x", x=12),
                                kdv, kdv, OP.mult)
        nc.vector.tensor_reduce(
            nk2[:].rearrange("p (h x) -> p h x", x=4),
            sqd[:].rearrange("p (h c x) -> p h x c", c=3, x=4), AX.X, OP.add)
        nc.vector.tensor_tensor(nq2[:], nq2[:], nk2[:], OP.mult)
        nc.scalar.activation(nq2[:], nq2[:], AF.Sqrt)
        nc.vector.tensor_scalar_add(nq2[:], nq2[:], EPS)
        nc.vector.reciprocal(nq2[:], nq2[:])
        nc.scalar.activation(csum[:], csum[:], AF.Sqrt)
        nc.vector.tensor_tensor(csum[:], csum[:], nq2[:], OP.mult)
        curv = tmp.tile([128, 12], F32, tag="curv", name="curv")
        nc.vector.tensor_reduce(
            curv[:], csum[:].rearrange("p (h x) -> p h x", x=4), AX.X, OP.add)
        # qf[41] = c2*q2s + c3*curv
        nc.vector.tensor_tensor(q2s[:], q2s[:], qcst[:, 108:120], OP.mult)
        nc.vector.tensor_tensor(curv[:], curv[:], qcst[:, 120:132], OP.mult)
        nc.vector.tensor_tensor(qfv[:, :, 41], q2s[:], curv[:], OP.add)

    # ---- qfT: masked transposes (even head | odd head halves) -----------
    qfT = [work.tile([128, 2 * NB], BF16, name=f"qfT{t}") for t in range(6)]
    for t in range(6):
        nc.gpsimd.memset(qfT[t][:], 0.0)
    for t in range(6):
        for qt in range(2):
            tps = tpsum.tile([128, 384], F32, tag="tps")
            nc.tensor.transpose(tps[:, 0:128],
                                qf_sb[qt][:, t * 128:(t + 1) * 128], ident[:])
            eng = nc.scalar if (t + qt) % 2 else nc.vector
            (eng.copy if eng is nc.scalar else eng.tensor_copy)(
                qfT[t][0:FS, qt * 128:(qt + 1) * 128], tps[0:FS, 0:128])
            eng2 = nc.vector if (t + qt) % 2 else nc.scalar
            (eng2.copy if eng2 is nc.scalar else eng2.tensor_copy)(
                qfT[t][64:64 + FS, NB + qt * 128:NB + (qt + 1) * 128],
                tps[64:64 + FS, 0:128])

    # ---- attention QK+exp, folded into the remaining key-tile work ------
    att_ctx = ExitStack()
    apsum = pre_ctx.enter_context(tc.tile_pool(name="apsum", bufs=2, space=PS))
    expT_tiles = [work.tile([128, 4096], BF16, name=f"expT{i}")
                  for i in range(6)]
    o_all = [work.tile([128, FEAT * H], F32, name=f"oall{qt}")
             for qt in range(2)]
    feats = [work.tile([128, FOUT], F32, name=f"feats{qt}") for qt in range(2)]
    ld_sb = [work.tile([128, 288], F32, name=f"ld{qt}") for qt in range(2)]

    def emit_qk_kbp(p4):
        for t in range(6):
            aps = apsum.tile([128, 1024], F32, tag="aps", name="aps")
            for j in range(2):
                kb = p4 * 2 + j
                nc.tensor.matmul(aps[:, j * 512:(j + 1) * 512],
                                 kfT3[:, t, kb * 128:(kb + 1) * 128],
                                 qfT[t][:, :], start=True, stop=True)
            nc.scalar.activation(expT_tiles[t][:, p4 * 1024:(p4 + 1) * 1024],
                                 aps[:], AF.Exp)

    emit_qk_kbp(0)
    for _kt in range(2, NKT):
        emit_ktile(_kt)
        if _kt % 2:
            emit_qk_kbp((_kt - 1) // 2)
    emit_wout_dmas()
    pre_ctx.close()
    opsum = att_ctx.enter_context(tc.tile_pool(name="opsum", bufs=2, space=PS))
    otp = att_ctx.enter_context(tc.tile_pool(name="otp", bufs=2, space=PS))

    def emit_av(h):
        t, e = h // 2, h % 2
        expT = expT_tiles[t]
        ot_ps = opsum.tile([OCH, NB], F32, tag="ot", name="ot_ps")
        for kb in range(NKT):
            nc.tensor.matmul(
                ot_ps[:], vaG[kb][:, h * OCH:(h + 1) * OCH],
                expT[:, kb * 512 + e * NB:kb * 512 + (e + 1) * NB],
                start=(kb == 0), stop=(kb == NKT - 1))
        ot_sb = tmp.tile([OCH, NB], F32R, tag="otsb", name="otsb", bufs=2)
        nc.vector.tensor_copy(ot_sb[:], ot_ps[:])
        for qt in range(2):
            tp = otp.tile([128, OCH], F32R, tag="tp", name="tp")
            nc.tensor.transpose(tp[:], ot_sb[:, qt * 128:(qt + 1) * 128],
                                ident_r[:, :])
            rec = tmp.tile([128, 1], F32, tag="rec", name="rec", bufs=2)
            nc.vector.reciprocal(rec[:], tp[:, 64:65].bitcast(F32))
            nc.vector.tensor_scalar_mul(
                o_all[qt][:, h * FEAT:h * FEAT + 64], tp[:, 0:64].bitcast(F32),
                rec[:])

    def emit_inv_rot(qt, hh):
        """Rotate o_geom back to local frame for heads hh*6..hh*6+5."""
        hs = slice(hh * 6, hh * 6 + 6)
        ov = o_all[qt][:].rearrange("p (h f) -> p h f", f=FEAT)[:, hs]
        gv = feats[qt][:, 192:FOUT].rearrange(
            "p (h x c) -> p h x c", h=H, c=7)[:, hs]

        def og(j):
            return ov[:, :, 16 + 16 * j:24 + 16 * j]

        ogs = tmp.tile([128, 144], F32, tag="ogs", name="ogs", bufs=2)
        ogs3 = ogs[:].rearrange("p (c x) -> p c x", c=3)
        for j in range(3):
            nc.vector.tensor_scalar(
                ogs3[:, j].rearrange("p (h x) -> p h x", x=V), og(j),
                Tc(qt, j), None, OP.subtract)
        lci = tmp.tile([128, 48], F32, tag="lci", name="lci", bufs=2)
        for i in range(3):
            nc.vector.tensor_scalar_mul(lci[:], ogs3[:, 0], Rc(qt, i))
            nc.vector.scalar_tensor_tensor(lci[:], ogs3[:, 1],
                                           Rc(qt, 3 + i), lci[:],
                                           OP.mult, OP.add)
            nc.vector.scalar_tensor_tensor(
                gv[:, :, :, i], ogs3[:, 2].rearrange("p (h x) -> p h x", x=V),
                Rc(qt, 6 + i),
                lci[:].rearrange("p (h x) -> p h x", x=V), OP.mult, OP.add)
            ldd = ld_sb[qt][:, i * 96 + hh * 48:i * 96 + (hh + 1) * 48]
            ldd3 = ldd.rearrange("p (h x) -> p h x", x=V)
            nc.vector.tensor_scalar_mul(ldd3, ov[:, :, 24:32], Rc(qt, i))
            nc.vector.scalar_tensor_tensor(ldd3, ov[:, :, 40:48],
                                           Rc(qt, 3 + i), ldd3,
                                           OP.mult, OP.add)
            nc.vector.scalar_tensor_tensor(ldd3, ov[:, :, 56:64],
                                           Rc(qt, 6 + i), ldd3,
                                           OP.mult, OP.add)

    def emit_inv_norm(qt):
        gv = feats[qt][:, 192:FOUT].rearrange("p (h x c) -> p h x c", h=H, c=7)
        lsq = tmp.tile([128, 288], F32, tag="lsq", name="lsq")
        lsq4 = lsq[:].rearrange("p (h x c) -> p h x c", c=3, x=V)
        nc.vector.tensor_tensor(lsq4, gv[:, :, :, 0:3], gv[:, :, :, 0:3],
                                OP.mult)
        ncs = tmp.tile([128, 96], F32, tag="ncs", name="ncs")
        nc.vector.tensor_reduce(
            ncs[:], lsq[:].rearrange("p (x c) -> p x c", c=3), AX.X, OP.add)
        nc.scalar.activation(gv[:, :, :, 6],
                             ncs[:].rearrange("p (h x) -> p h x", x=V), AF.Sqrt)
        # ld normalization
        ldq = ld_sb[qt]
        nc.gpsimd.tensor_tensor(lsq[:], ldq[:], ldq[:], OP.mult)
        nds = tmp.tile([128, 96], F32, tag="nds", name="nds")
        nc.vector.tensor_reduce(
            nds[:], lsq[:].rearrange("p (c x) -> p x c", c=3), AX.X, OP.add)
        nc.scalar.activation(nds[:], nds[:], AF.Sqrt)
        nc.vector.tensor_scalar_max(nds[:], nds[:], EPS)
        nc.vector.reciprocal(nds[:], nds[:])
        nds3 = nds[:].rearrange("p (h x) -> p h x", x=V)
        for i in range(3):
            nc.gpsimd.tensor_tensor(
                gv[:, :, :, 3 + i],
                ldq[:, i * 96:(i + 1) * 96].rearrange("p (h x) -> p h x", x=V),
                nds3, OP.mult)
        nc.gpsimd.tensor_copy(
            feats[qt][:, 0:192].rearrange("p (h c) -> p h c", c=16),
            o_all[qt][:].rearrange("p (h f) -> p h f", f=FEAT)[:, :, 0:16])

    for h in range(H):
        emit_av(h)
        if h == 5:
            emit_inv_rot(0, 0)
            emit_inv_rot(1, 0)
    # ---- inverse norms + output projection, pipelined per query tile -----
    att_ctx.close()
    tpsum2 = ctx.enter_context(tc.tile_pool(name="tpsum2", bufs=2, space=PS))
    opsum2 = ctx.enter_context(tc.tile_pool(name="opsum2", bufs=2, space=PS))
    fT = []
    for kc in range(KCH):
        r0 = kc * 128
        rw = min(FOUT, r0 + 128) - r0
        pw = rw + 2 if kc == KCH - 1 else rw
        fT.append(work.tile([pw, NB], F32R, name=f"fT{kc}"))
    lastr = FOUT - (KCH - 1) * 128
    nc.gpsimd.tensor_copy(fT[KCH - 1][lastr:lastr + 2, :], ones2_f32[:])
    for qt in range(2):
        emit_inv_rot(qt, 1)
        emit_inv_norm(qt)
        for kc in range(KCH):
            r0 = kc * 128
            rw = min(FOUT, r0 + 128) - r0
            ps = tpsum2.tile([128, 128], F32, tag="tps2")
            nc.tensor.transpose(ps[:rw, :], feats[qt][:, r0:r0 + rw], ident[:])
            if kc % 2:
                nc.scalar.copy(fT[kc][:rw, qt * 128:(qt + 1) * 128], ps[:rw, :])
            else:
                nc.vector.tensor_copy(fT[kc][:rw, qt * 128:(qt + 1) * 128],
                                      ps[:rw, :])
        ps = opsum2.tile([128, CS], F32, tag="oproj")
        for kc in range(KCH):
            nc.tensor.matmul(ps[:], fT[kc][:, qt * 128:(qt + 1) * 128],
                             wout_sb[kc][:], start=(kc == 0),
                             stop=(kc == KCH - 1))
        osb = tmp.tile([128, CS], F32, tag="osb", name="osb")
        nc.scalar.copy(osb[:], ps[:])
        nc.sync.dma_start(out_loc[qt * 128:(qt + 1) * 128, :], osb[:])


def _run(inputs, trace=False):
    s, rt_all, wall, wout_b, qconst, has_bias = _host_prep(inputs)
    nc = _build_program(has_bias)
    in_maps = []
    for c in range(8):
        b, qb = c // 4, c % 4
        # rotate key rows so this core's queries are rows 0:256
        idx = np.r_[qb * NB:N, 0:qb * NB]
        in_maps.append({
            "s_all": np.ascontiguousarray(s[b][idx]),
            "rt_all": np.ascontiguousarray(rt_all[b][idx]),
            "wall": wall, "wout_b": wout_b, "qconst": qconst,
        })
    res = run_bass_kernel_spmd(nc, in_maps, list(range(8)), trace=trace)
    out = np.empty((B, N, CS), np.float32)
    for c in range(8):
        b, qb = c // 4, c % 4
        out[b, qb * NB:(qb + 1) * NB] = res.results[c]["out_loc"]
    return out, res


def kernel(**inputs):
    out, _ = _run(inputs, trace=False)
    return out


def kernel_traced(**inputs):
    return _run(inputs, trace=True)
